# revision 2
# baseline (speedup 1.0000x reference)
"""Trainium2 Bass kernel for DanceDecoder: 2-layer autoregressive LSTM.

B=8192, T=60, HID=512, OUT=51, LAT=64.  Data-parallel over 8 cores
(1024 batch rows each).  Feature-major layout: features on SBUF
partitions, batch in the free dimension (2 blocks of 512 columns).

The recurrent matmuls (h @ W_hh etc., 96% of the FLOPs) run in fp8
e4m3 with MatmulPerfMode.DoubleRow (two K-tiles of 128 per
instruction, 2 rows/cycle): weights are pre-scaled by 4096 host-side
so they stay in fp8 normal range, and the 1/4096 descale is folded
into the activation `scale` operand.  The x feed-back term and the
fc_init/fc_out projections stay f32r/bf16.  Gate activations are
staged in bf16 SBUF so the DVE elementwise ops hit the 2x/4x packed
perf modes; the c state is bf16, h state fp8 (plus a bf16 copy of h2
for the fc_out readout).  PSUM: one [128,2,512] tile per gate (both
batch blocks side by side) -> 1024-wide activation instructions.
"""
import sys
sys.path.insert(0, "/opt/trn_rl_repo")

import numpy as np
import ml_dtypes
import concourse.bacc as bacc
import concourse.mybir as mybir
import concourse.tile as tile
from concourse.bass_utils import run_bass_kernel_spmd

HID = 512
OUT = 51
LAT = 64
T = 60
B = 8192
NCORES = 8
BC = B // NCORES          # 1024 batch columns per core
NBLK = 2                  # column blocks per core
NB = BC // NBLK           # 512 columns per block
KH = HID // 128           # 4 hidden chunks

WS = 4096.0               # fp8 weight pre-scale
DS = 1.0 / WS             # descale applied in the activation

F32 = mybir.dt.float32
F32R = mybir.dt.float32r
BF16 = mybir.dt.bfloat16
F8 = mybir.dt.float8e4
AF = mybir.ActivationFunctionType
OP = mybir.AluOpType
DRow = mybir.MatmulPerfMode.DoubleRow

_cached = {}


def build_module(unroll_T=T, debug_taps=False, repeat=1):
    nc = bacc.Bacc("TRN2", target_bir_lowering=False, debug=False)

    # ---- DRAM I/O (per core) ----
    zT = nc.dram_tensor("zT", [LAT, BC], F32R, kind="ExternalInput")
    x0 = nc.dram_tensor("x0", [OUT, BC], F32R, kind="ExternalInput")
    wih0T = nc.dram_tensor("wih0T", [OUT, 4 * HID], F32R, kind="ExternalInput")
    whh0T8 = nc.dram_tensor("whh0T8", [HID, 4 * HID], F8, kind="ExternalInput")
    w1T8 = nc.dram_tensor("w1T8", [2 * HID, 4 * HID], F8, kind="ExternalInput")
    foutT = nc.dram_tensor("foutT", [HID, OUT], BF16, kind="ExternalInput")
    finitT = nc.dram_tensor("finitT", [LAT, 2 * HID], F32R, kind="ExternalInput")
    b0c = nc.dram_tensor("b0c", [128, 16], F32, kind="ExternalInput")
    b1c = nc.dram_tensor("b1c", [128, 16], F32, kind="ExternalInput")
    binitc = nc.dram_tensor("binitc", [128, 2 * KH], F32, kind="ExternalInput")
    boutc = nc.dram_tensor("boutc", [OUT, 1], F32, kind="ExternalInput")
    frames = nc.dram_tensor("frames", [unroll_T, OUT, BC], F32,
                            kind="ExternalOutput")

    with tile.TileContext(nc) as tc:
        with (
            tc.tile_pool(name="wpool", bufs=1) as wp,
            tc.tile_pool(name="spool", bufs=1) as sp,
            tc.tile_pool(name="tmp", bufs=2) as tp,
            tc.tile_pool(name="psum", bufs=1, space="PSUM") as pp,
        ):
            # ---- persistent SBUF tiles ----
            w_ih0 = wp.tile([OUT, 4 * HID], F32R, tag="w_ih0")
            w_hh0 = wp.tile([128, KH, 4 * HID], F8, tag="w_hh0")
            w_1 = wp.tile([128, 2 * KH, 4 * HID], F8, tag="w_1")
            w_out = wp.tile([128, KH, OUT], BF16, tag="w_out")
            w_init = wp.tile([LAT, 2 * HID], F32R, tag="w_init")
            bias0 = wp.tile([128, 16], F32, tag="bias0")
            bias1 = wp.tile([128, 16], F32, tag="bias1")
            biasi = wp.tile([128, 2 * KH], F32, tag="biasi")
            biaso = wp.tile([OUT, 1], F32, tag="biaso")
            z_t = wp.tile([LAT, BC], F32R, tag="z_t")
            x = wp.tile([OUT, NBLK, NB], F32R, tag="x")

            # ping-pong h state: [t%2] read, [1 - t%2] written
            h1_8 = [sp.tile([128, KH, NBLK, NB], F8, tag=f"h1_8{p}",
                            name=f"h1_8{p}") for p in range(2)]
            h2_8 = [sp.tile([128, KH, NBLK, NB], F8, tag=f"h2_8{p}",
                            name=f"h2_8{p}") for p in range(2)]
            h2_bf = sp.tile([128, KH, NBLK, NB], BF16, tag="h2_bf")
            c1 = sp.tile([128, KH, NBLK, NB], BF16, tag="c1")
            c2 = sp.tile([128, KH, NBLK, NB], BF16, tag="c2")

            # PSUM: one [128, 2 blocks, 512] tile per gate = 2 banks x4 = 8
            P = {g: pp.tile([128, NBLK, NB], F32, tag=f"P{g}", name=f"P{g}")
                 for g in "ifgo"}
            GI = {"i": 0, "f": 1, "g": 2, "o": 3}

            # ---- load everything ----
            nc.sync.dma_start(w_ih0[:], wih0T[:])
            for j in range(KH):
                nc.sync.dma_start(w_hh0[:, j, :], whh0T8[j * 128:(j + 1) * 128, :])
                nc.sync.dma_start(w_out[:, j, :], foutT[j * 128:(j + 1) * 128, :])
            for j in range(2 * KH):
                nc.sync.dma_start(w_1[:, j, :], w1T8[j * 128:(j + 1) * 128, :])
            nc.sync.dma_start(w_init[:], finitT[:])
            nc.sync.dma_start(bias0[:], b0c[:])
            nc.sync.dma_start(bias1[:], b1c[:])
            nc.sync.dma_start(biasi[:], binitc[:])
            nc.sync.dma_start(biaso[:], boutc[:])
            nc.sync.dma_start(z_t[:], zT[:])
            nc.sync.dma_start(x[:, 0, :], x0[:, 0:NB])
            nc.sync.dma_start(x[:, 1, :], x0[:, NB:BC])

            # ---- init: h0/c0 = fc_init(z), replicated into both layers ----
            gtags = "ifgo"
            for b in range(NBLK):
                s = b * NB
                for m in range(2 * KH):
                    acc = P[gtags[m % 4]][:, b, :]
                    nc.tensor.matmul(acc,
                                     w_init[:, m * 128:(m + 1) * 128],
                                     z_t[:, s:s + NB],
                                     start=True, stop=True)
                    if m < KH:
                        nc.vector.tensor_scalar(h1_8[0][:, m, b, :], acc,
                                                biasi[:, m:m + 1], None, OP.add)
                        nc.vector.tensor_copy(h2_8[0][:, m, b, :],
                                              h1_8[0][:, m, b, :])
                    else:
                        nc.vector.tensor_scalar(c1[:, m - KH, b, :], acc,
                                                biasi[:, m:m + 1], None, OP.add)
                        nc.vector.tensor_copy(c2[:, m - KH, b, :],
                                              c1[:, m - KH, b, :])

            # ---- one LSTM cell chunk: matmuls into PSUM, then gate math ----
            def cell(k, layer, h1p, h1n, h2p, bias):
                """Emit PE+Act+DVE(+Pool) for chunk k of `layer`."""
                c_st = c1 if layer == 0 else c2
                for g in "igfo":
                    col = GI[g] * HID + k * 128
                    for b in range(NBLK):
                        acc = P[g][:, b, :]
                        if layer == 0:
                            nc.tensor.matmul(acc, w_hh0[:, 0:2, col:col + 128],
                                             h1p[:, 0:2, b, :],
                                             start=True, stop=False,
                                             perf_mode=DRow)
                            nc.tensor.matmul(acc, w_hh0[:, 2:4, col:col + 128],
                                             h1p[:, 2:4, b, :],
                                             start=False, stop=False,
                                             perf_mode=DRow)
                            nc.tensor.matmul(acc, w_ih0[:, col:col + 128],
                                             x[:, b, :],
                                             start=False, stop=True)
                        else:
                            nc.tensor.matmul(acc, w_1[:, 0:2, col:col + 128],
                                             h1n[:, 0:2, b, :],
                                             start=True, stop=False,
                                             perf_mode=DRow)
                            nc.tensor.matmul(acc, w_1[:, 2:4, col:col + 128],
                                             h1n[:, 2:4, b, :],
                                             start=False, stop=False,
                                             perf_mode=DRow)
                            nc.tensor.matmul(acc, w_1[:, 4:6, col:col + 128],
                                             h2p[:, 0:2, b, :],
                                             start=False, stop=False,
                                             perf_mode=DRow)
                            nc.tensor.matmul(acc, w_1[:, 6:8, col:col + 128],
                                             h2p[:, 2:4, b, :],
                                             start=False, stop=True,
                                             perf_mode=DRow)
                i_sb = tp.tile([128, NBLK, NB], BF16, tag="i_sb")
                f_sb = tp.tile([128, NBLK, NB], BF16, tag="f_sb")
                g_sb = tp.tile([128, NBLK, NB], BF16, tag="g_sb")
                o_sb = tp.tile([128, NBLK, NB], BF16, tag="o_sb")
                t_sb = tp.tile([128, NBLK, NB], BF16, tag="t_sb")
                cs = c_st[:, k, :, :]
                nc.scalar.activation(i_sb[:], P["i"][:], AF.Sigmoid,
                                     bias=bias[:, k:k + 1], scale=DS)
                nc.scalar.activation(g_sb[:], P["g"][:], AF.Tanh,
                                     bias=bias[:, 8 + k:8 + k + 1], scale=DS)
                nc.vector.tensor_tensor(g_sb[:], i_sb[:], g_sb[:], OP.mult)
                nc.scalar.activation(f_sb[:], P["f"][:], AF.Sigmoid,
                                     bias=bias[:, 4 + k:4 + k + 1], scale=DS)
                nc.vector.tensor_tensor(cs, f_sb[:], cs, OP.mult)
                nc.vector.tensor_tensor(cs, cs, g_sb[:], OP.add)
                nc.scalar.activation(o_sb[:], P["o"][:], AF.Sigmoid,
                                     bias=bias[:, 12 + k:12 + k + 1], scale=DS)
                nc.scalar.activation(t_sb[:], cs, AF.Tanh)
                if layer == 0:
                    nc.vector.tensor_tensor(h1n[:, k, :, :], o_sb[:], t_sb[:],
                                            OP.mult)
                else:
                    nc.vector.tensor_tensor(h2_bf[:, k, :, :], o_sb[:],
                                            t_sb[:], OP.mult)

            # ---- the autoregressive steps ----
            for t in range(unroll_T):
                p = t % 2
                h1p, h1n = h1_8[p], h1_8[1 - p]
                h2p, h2n = h2_8[p], h2_8[1 - p]
                for k in range(KH):
                    cell(k, 0, h1p, h1n, h2p, bias0)
                for k in range(KH):
                    cell(k, 1, h1p, h1n, h2p, bias1)
                    nc.gpsimd.tensor_copy(h2n[:, k, :, :], h2_bf[:, k, :, :])
                for b in range(NBLK):
                    acc = P["g"][0:OUT, b, :]
                    for j in range(KH):
                        nc.tensor.matmul(acc, w_out[:, j, :],
                                         h2_bf[:, j, b, :],
                                         start=(j == 0), stop=(j == KH - 1))
                    nc.vector.tensor_scalar(x[:, b, :], acc, biaso[:],
                                            None, OP.add)
                    nc.sync.dma_start(frames[t, :, b * NB:(b + 1) * NB],
                                      x[:, b, :].bitcast(F32))

    nc.compile()
    return nc


def _prep_inputs(z, start_token, fc_init_w, fc_init_b,
                 w_ih0, w_hh0, b_ih0, b_hh0,
                 w_ih1, w_hh1, b_ih1, b_hh1,
                 fc_out_w, fc_out_b):
    f32 = np.float32
    f8 = ml_dtypes.float8_e4m3
    bf = ml_dtypes.bfloat16
    common = {
        "wih0T": np.ascontiguousarray(w_ih0.T * WS, dtype=f32),
        "whh0T8": np.ascontiguousarray(w_hh0.T * WS).astype(f8),
        "w1T8": np.ascontiguousarray(
            np.concatenate([w_ih1.T, w_hh1.T], axis=0) * WS).astype(f8),
        "foutT": np.ascontiguousarray(fc_out_w.T).astype(bf),
        "finitT": np.ascontiguousarray(fc_init_w.T, dtype=f32),
        "b0c": np.ascontiguousarray(
            (b_ih0 + b_hh0).reshape(4, 4, 128).transpose(2, 0, 1)
            .reshape(128, 16), dtype=f32),
        "b1c": np.ascontiguousarray(
            (b_ih1 + b_hh1).reshape(4, 4, 128).transpose(2, 0, 1)
            .reshape(128, 16), dtype=f32),
        "binitc": np.ascontiguousarray(
            fc_init_b.reshape(2 * KH, 128).T, dtype=f32),
        "boutc": np.ascontiguousarray(fc_out_b[:, None], dtype=f32),
        "x0": np.ascontiguousarray(
            np.broadcast_to(start_token[:, None], (OUT, BC)), dtype=f32),
    }
    in_maps = []
    for c in range(NCORES):
        m = dict(common)
        m["zT"] = np.ascontiguousarray(
            z[c * BC:(c + 1) * BC].T, dtype=f32)
        in_maps.append(m)
    return in_maps


def kernel(**inputs):
    if "nc" not in _cached:
        _cached["nc"] = build_module()
    nc = _cached["nc"]
    in_maps = _prep_inputs(**inputs)
    res = run_bass_kernel_spmd(nc, in_maps, list(range(NCORES)))
    # frames per core: [T, OUT, BC] -> full [B, T, OUT]
    out = np.stack([res.results[c]["frames"] for c in range(NCORES)])
    return np.ascontiguousarray(
        out.transpose(0, 3, 1, 2).reshape(B, T, OUT))
